# revision 14
# baseline (speedup 1.0000x reference)
"""Weighted 2D cross-entropy (BCE-over-classes) loss on 8 Trainium2 cores.

Math (matches the reference):
  t in [0,19); pos = t>0, neg = t==0 (all pixels are pos or neg; mask == 1)
  S(i) = sum_c bce(i,c) = -lnR(i)
     lnR(i) = A(i) + B(i)
     A(i)   = sum_c ln(1-p_c(i))
     B(i)   = ln(p_t(i)) - ln(1-p_t(i)) = ln(e^{-lsel(i)} - 1),  lsel = ln(1-p_t)
  loss = ( (NEG/TOT)*S_pos_sum + (POS/TOT)*S_neg_sum ) / (TOT*C)

Per-core (core k <- batch element k, pure data parallel), FOUR quarter-passes
over pixel quarters [128, 1024].  A quarter's PSUM accumulators (A + lsel)
occupy 4 banks, so two quarters ping-pong in PSUM: while quarter q's tail
(Exp/Ln/STT chain on ACT+DVE) drains its PSUM banks, the PE already streams
quarter q+1's matmuls -- no half-boundary stall.
Classes are processed in PAIRS per quarter: one 3D DMA brings both classes'
quarter into a [128, 2, 1024] tile, so ACT runs ONE 2048-wide Ln and DVE ONE
2048-wide mult per pair -- the ~294ns fixed per-activation overhead halves
versus 1024-wide ops (ACT is the pipeline pacer once DMA stalls are gone).
  - per pair: 1MB DMA of p, ACT Ln(1-p)->bf16, DVE eq=(t==c) (4x, x2) and
    masked=eq*L (2x), PE identity-matmuls accumulate A and lsel in PSUM.
  - tail per quarter: expm=Exp(-lsel); B=Ln(expm-1) (fused -1 bias via a
    [128,1] const column, no DVE subtract); lnR=B+A via STT with accum_out;
    pos-masked sum via a second STT accum.
  - the unpaired class 18 runs per-quarter; in the last quarter it and the
    tail run in 512-wide chunks so the post-last-DMA drain is short.
Target is converted to bf16 on HOST (1MB instead of 2MB int32 DMA, no
on-chip CAST, and the first predict tile lands sooner).
Activation tables are pinned to natural_log_exp_and_others (holds both
ln and exp) -- otherwise bacc's table-load pass alternates between the
ln-only and exp-only sets, paying ~1.3us per reload.
Counts (pos/neg) are computed on host from the int target directly.
Per-core output is the raw [128, 16] per-partition stats; the final
partition reduce + 8-way combine happens on host in float64.
"""

from contextlib import ExitStack

import numpy as np

import concourse.bass as bass
import concourse.mybir as mybir
import concourse.tile as tile
from concourse import bacc
from concourse.bass_utils import run_bass_kernel_spmd

# problem shape (hardcoded per harness contract)
N, C, H, W = 8, 19, 512, 1024
PIX = H * W          # 524288 pixels per core
P = 128              # partitions
FCOLS = PIX // P     # 4096 free columns when pixels laid out [128, 4096]
QW = FCOLS // 4      # 1024: quarter width
HQW = QW // 2        # 512: final-chunk / matmul width
NPAIR = C // 2       # 9 class pairs; class 18 is the unpaired tail class
N_CORES = 8
NSTAT = 16           # stats columns in the [128, 16] output

DT = mybir.dt

# stats column layout ([128, 16] f32; host folds):
#   0-2 : sum lnR      for quarters 0-2
#   3-4 : sum lnR      for quarter 3 chunks 0,1
#   8-10: sum pos*lnR  for quarters 0-2
#   11-12: sum pos*lnR for quarter 3 chunks 0,1
COL_LNR = 0
COL_POSLNR = 8

_ACT_TABLES_PATCHED = False


def _pin_act_table_set():
    """Restrict Ln/Exp to the natural_log_exp_and_others set so bacc's
    table-load pass emits a single ACT_TABLE_LOAD instead of thrashing
    between the ln-only and exp-only sets (~1.3us per reload).  Set
    indices must stay aligned with act_info.json, so every set entry is
    kept -- only the Ln/Exp membership of the other sets is dropped."""
    global _ACT_TABLES_PATCHED
    if _ACT_TABLES_PATCHED:
        return
    import concourse.bacc as bacc_mod

    orig = bacc_mod.get_activation_tables
    ln_exp = {mybir.ActivationFunctionType.Ln, mybir.ActivationFunctionType.Exp}

    def patched(arch):
        tables = orig(arch)
        return {
            name: (fns if name == "natural_log_exp_and_others" else fns - ln_exp)
            for name, fns in tables.items()
        }

    bacc_mod.get_activation_tables = patched
    _ACT_TABLES_PATCHED = True


def build_kernel() -> bass.Bass:
    _pin_act_table_set()

    # Bacc (not raw Bass): its compile() pipeline runs
    # generate_event_semaphores, which splits multi-sem waits to satisfy the
    # 1-wait-per-instruction TRN2 sync structs -- raw Bass modules with
    # Tile-emitted multi-waits fail walrus codegen.
    nc = bacc.Bacc("TRN2")

    predict = nc.declare_dram_parameter("predict", [C, PIX], DT.float32, isOutput=False)
    target = nc.declare_dram_parameter("target", [P, FCOLS], DT.bfloat16, isOutput=False)
    idn = nc.declare_dram_parameter("idn", [P, P], DT.bfloat16, isOutput=False)
    out = nc.declare_dram_parameter("out", [P, NSTAT], DT.float32, isOutput=True)

    pred_r = predict.rearrange("c (p f) -> c p f", p=P)  # [19, 128, 4096]
    pred_t = predict.rearrange("c (p f) -> p c f", p=P)  # [128, 19, 4096]

    with tile.TileContext(nc) as tc, ExitStack() as ctx:
        const = ctx.enter_context(tc.tile_pool(name="const", bufs=1))
        # p bufs=8 aligns slot reuse with the global DMA->DMAHW-proc
        # round-robin (8 procs), so the WAW on the old writer is same-proc
        # FIFO order and Tile emits no cross-queue wait
        p_pool = ctx.enter_context(tc.tile_pool(name="p", bufs=8))
        lm_pool = ctx.enter_context(tc.tile_pool(name="lm", bufs=4))
        eq_pool = ctx.enter_context(tc.tile_pool(name="eq", bufs=4))
        # class-18 (unpaired) tiles are smaller and only 1 per quarter
        p18_pool = ctx.enter_context(tc.tile_pool(name="p18", bufs=4))
        lm18_pool = ctx.enter_context(tc.tile_pool(name="lm18", bufs=2))
        eq18_pool = ctx.enter_context(tc.tile_pool(name="eq18", bufs=2))
        tail_pool = ctx.enter_context(tc.tile_pool(name="tail", bufs=2))
        psA_pool = ctx.enter_context(tc.tile_pool(name="psA", bufs=2, space="PSUM"))
        psL_pool = ctx.enter_context(tc.tile_pool(name="psL", bufs=2, space="PSUM"))

        t_bf = const.tile([P, FCOLS], DT.bfloat16, tag="tb")
        # quarter 0 of target first so the q0 eq chain is ready before p0
        nc.sync.dma_start(out=t_bf[:, 0:QW], in_=target[:, 0:QW])

        idn_sb = const.tile([P, P], DT.bfloat16, tag="idn")
        stats = const.tile([P, NSTAT], DT.float32, tag="stats")
        # per-partition -1.0 bias column for the fused Ln(expm - 1) tail
        negone = const.tile([P, 1], DT.float32, tag="negone")

        state = {"n_dma": 0}

        def post_pair_dma():
            # constants queue behind pair0's DMAs; the bulk of target
            # queues behind pair1 -- the q0 pipeline primes first
            state["n_dma"] += 1
            if state["n_dma"] == 1:
                nc.sync.dma_start(out=idn_sb[:], in_=idn[:])
                nc.vector.memset(stats[:], 0.0)
                nc.vector.memset(negone[:], -1.0)
            elif state["n_dma"] == 2:
                nc.sync.dma_start(out=t_bf[:, QW:], in_=target[:, QW:])

        def emit_tail(q, a_ps, l_ps):
            # tail: B = Ln(e^{-lsel} - 1) (bias fuses the -1); lnR = B + A.
            # quarter 3 drains in two 512 chunks to shorten the final
            # latency chain after the last DMA byte.
            qbase = q * QW
            tail_chunks = [(s * HQW, HQW) for s in range(2)] if q == 3 else [(0, QW)]
            for ci, (toff, twidth) in enumerate(tail_chunks):
                qsl = slice(toff, toff + twidth)
                col = q + ci if q < 3 else 3 + ci
                expm = tail_pool.tile([P, QW], DT.float32, tag="expm")
                nc.scalar.activation(
                    out=expm[:, :twidth],
                    in_=l_ps[:, qsl],
                    func=mybir.ActivationFunctionType.Exp,
                    scale=-1.0,
                )
                bb = tail_pool.tile([P, QW], DT.float32, tag="bb")
                nc.scalar.activation(
                    out=bb[:, :twidth],
                    in_=expm[:, :twidth],
                    func=mybir.ActivationFunctionType.Ln,
                    bias=negone[:],
                )
                lnr = tail_pool.tile([P, QW], DT.float32, tag="lnr")
                nc.vector.scalar_tensor_tensor(
                    out=lnr[:, :twidth],
                    in0=bb[:, :twidth],
                    scalar=0.0,
                    in1=a_ps[:, qsl],
                    op0=mybir.AluOpType.add,
                    op1=mybir.AluOpType.add,
                    accum_out=stats[:, COL_LNR + col : COL_LNR + col + 1],
                )
                scr = tail_pool.tile([P, QW], DT.float32, tag="scr")
                nc.vector.scalar_tensor_tensor(
                    out=scr[:, :twidth],
                    in0=t_bf[:, qbase + toff : qbase + toff + twidth],
                    scalar=0.5,
                    in1=lnr[:, :twidth],
                    op0=mybir.AluOpType.is_gt,
                    op1=mybir.AluOpType.mult,
                    accum_out=stats[:, COL_POSLNR + col : COL_POSLNR + col + 1],
                )

        # tail(q) is deferred into quarter q+1 (after its pair 1): the Exp
        # depends on q's final lsel matmul, and ACT is in-order -- emitting
        # the tail at the quarter boundary stalls ACT ~1.8us per quarter
        pending_tail = None

        for q in range(4):
            qbase = q * QW
            qsl_full = slice(qbase, qbase + QW)
            # PSUM accumulators for this quarter (ping-pong, 2+2 banks each)
            a_ps = psA_pool.tile([P, QW], DT.float32, tag="aps")
            l_ps = psL_pool.tile([P, QW], DT.float32, tag="lps")

            for pair in range(NPAIR):
                c = 2 * pair
                # two plain 2D DMAs (128 descriptors each) into one tile:
                # a single 3D [p, c, f] DMA costs ~2.2us of serial
                # descriptor-generation on the sync sequencer vs ~0.6us x2
                p_t = p_pool.tile([P, 2, QW], DT.float32, tag="p")
                nc.sync.dma_start(out=p_t[:, 0, :], in_=pred_r[c, :, qsl_full])
                post_pair_dma()
                nc.sync.dma_start(out=p_t[:, 1, :], in_=pred_r[c + 1, :, qsl_full])
                post_pair_dma()

                # lm[:, 0:2, :] = L = Ln(1-p) bf16 ; lm[:, 2:4, :] = (T==c)*L
                lm = lm_pool.tile([P, 4, QW], DT.bfloat16, tag="lm")
                nc.scalar.activation(
                    out=lm[:, 0:2, :],
                    in_=p_t[:, :, :],
                    func=mybir.ActivationFunctionType.Ln,
                    bias=1.0,
                    scale=-1.0,
                )
                # eq at DVE 4x (16-bit tensor_scalar) + mult at 2x beats
                # the fused scalar_tensor_tensor, which only has a 1x uop
                eq = eq_pool.tile([P, 2, QW], DT.bfloat16, tag="eq")
                for j in range(2):
                    nc.vector.tensor_scalar(
                        out=eq[:, j, :],
                        in0=t_bf[:, qsl_full],
                        scalar1=float(c + j),
                        scalar2=None,
                        op0=mybir.AluOpType.is_equal,
                    )
                nc.vector.tensor_mul(
                    out=lm[:, 2:4, :],
                    in0=eq[:, 0:2, :],
                    in1=lm[:, 0:2, :],
                )

                # lsel matmuls first: l_ps frees early in the tail (Exp
                # is its only reader), so the next quarter's PE work
                # restarts sooner
                for mrow, which in ((2, "l"), (3, "l"), (0, "a"), (1, "a")):
                    dst_ps = l_ps if which == "l" else a_ps
                    cc = c + (mrow % 2)
                    for s in range(2):
                        nc.tensor.matmul(
                            dst_ps[:, s * HQW : (s + 1) * HQW],
                            lhsT=idn_sb[:],
                            rhs=lm[:, mrow, s * HQW : (s + 1) * HQW],
                            start=(cc == 0),
                            stop=False,
                        )

                if pair == 1 and pending_tail is not None:
                    emit_tail(*pending_tail)
                    pending_tail = None

            # unpaired class 18; split into two 512 chunks in the last
            # quarter so the tail can start on chunk 0 while chunk 1
            # still computes
            chunks = [(s * HQW, HQW) for s in range(2)] if q == 3 else [(0, QW)]
            for off, width in chunks:
                csl = slice(qbase + off, qbase + off + width)
                p_s = p18_pool.tile([P, QW], DT.float32, tag="p18")
                nc.sync.dma_start(out=p_s[:, :width], in_=pred_r[C - 1, :, csl])
                lm = lm18_pool.tile([P, 2 * QW], DT.bfloat16, tag="lm18")
                nc.scalar.activation(
                    out=lm[:, :width],
                    in_=p_s[:, :width],
                    func=mybir.ActivationFunctionType.Ln,
                    bias=1.0,
                    scale=-1.0,
                )
                eq = eq18_pool.tile([P, QW], DT.bfloat16, tag="eq18")
                nc.vector.tensor_scalar(
                    out=eq[:, :width],
                    in0=t_bf[:, csl],
                    scalar1=float(C - 1),
                    scalar2=None,
                    op0=mybir.AluOpType.is_equal,
                )
                nc.vector.tensor_mul(
                    out=lm[:, QW : QW + width],
                    in0=eq[:, :width],
                    in1=lm[:, :width],
                )
                for s in range(width // HQW):
                    nc.tensor.matmul(
                        l_ps[:, off + s * HQW : off + (s + 1) * HQW],
                        lhsT=idn_sb[:],
                        rhs=lm[:, QW + s * HQW : QW + (s + 1) * HQW],
                        start=False,
                        stop=True,
                    )
                for s in range(width // HQW):
                    nc.tensor.matmul(
                        a_ps[:, off + s * HQW : off + (s + 1) * HQW],
                        lhsT=idn_sb[:],
                        rhs=lm[:, s * HQW : (s + 1) * HQW],
                        start=False,
                        stop=True,
                    )

            pending_tail = (q, a_ps, l_ps)

        # final quarter's tail runs at the very end (512-wide chunks)
        emit_tail(*pending_tail)

        nc.sync.dma_start(out=out[:], in_=stats[:])

    if not nc.is_finalized():
        nc.finalize()

    return nc


_NC_CACHE = None


def make_in_maps(predict: np.ndarray, target: np.ndarray):
    import ml_dtypes

    predict = np.ascontiguousarray(predict, dtype=np.float32)
    target_bf = np.ascontiguousarray(target, dtype=np.int32).astype(ml_dtypes.bfloat16)
    idn = np.eye(P, dtype=np.float32).astype(ml_dtypes.bfloat16)

    in_maps = []
    for k in range(N_CORES):
        in_maps.append(
            {
                "predict": predict[k].reshape(C, PIX),
                "target": target_bf[k].reshape(P, FCOLS),
                "idn": idn,
            }
        )
    return in_maps


def combine_host(results, target: np.ndarray) -> np.float32:
    tot = np.float64(0.0)
    s_all = np.float64(0.0)
    s_pos = np.float64(0.0)
    for k in range(N_CORES):
        st = results[k]["out"].reshape(P, NSTAT).astype(np.float64)
        s_all += -np.sum(st[:, COL_LNR : COL_LNR + 5])
        s_pos += -np.sum(st[:, COL_POSLNR : COL_POSLNR + 5])
        tot += PIX
    pos = np.float64(np.count_nonzero(target))
    neg = tot - pos
    s_neg = s_all - s_pos
    loss = ((neg / tot) * s_pos + (pos / tot) * s_neg) / (tot * C)
    return np.float32(loss)


def kernel(predict: np.ndarray, target: np.ndarray) -> np.ndarray:
    global _NC_CACHE
    if _NC_CACHE is None:
        _NC_CACHE = build_kernel()
    nc = _NC_CACHE

    in_maps = make_in_maps(predict, target)
    res = run_bass_kernel_spmd(nc, in_maps, list(range(N_CORES)))
    return combine_host(res.results, target)


# revision 17
# speedup vs baseline: 1.1254x; 1.1254x over previous
"""Weighted 2D cross-entropy (BCE-over-classes) loss on 8 Trainium2 cores.

Math (matches the reference):
  t in [0,19); pos = t>0, neg = t==0 (all pixels are pos or neg; mask == 1)
  S(i) = sum_c bce(i,c) = -lnR(i)
     lnR(i) = A(i) + B(i)
     A(i)   = sum_c ln(1-p_c(i))
     B(i)   = ln(p_t(i)) - ln(1-p_t(i)) = ln(e^{-lsel(i)} - 1),  lsel = ln(1-p_t)
  loss = ( (NEG/TOT)*S_pos_sum + (POS/TOT)*S_neg_sum ) / (TOT*C)

Per-core (core k <- batch element k, pure data parallel), FOUR quarter-passes
over pixel quarters [128, 1024].  A quarter's PSUM accumulators (A + lsel)
occupy 4 banks, so two quarters ping-pong in PSUM: while quarter q's tail
(Exp/Ln/STT chain on ACT+DVE) drains its PSUM banks, the PE already streams
quarter q+1's matmuls -- no half-boundary stall.
Classes are processed in PAIRS per quarter: one 3D DMA brings both classes'
quarter into a [128, 2, 1024] tile, so ACT runs ONE 2048-wide Ln and DVE ONE
2048-wide mult per pair -- the ~294ns fixed per-activation overhead halves
versus 1024-wide ops (ACT is the pipeline pacer once DMA stalls are gone).
  - per pair: 1MB DMA of p, ACT Ln(1-p)->bf16, DVE eq=(t==c) (4x, x2) and
    masked=eq*L (2x), PE identity-matmuls accumulate A and lsel in PSUM.
  - tail per quarter: expm=Exp(-lsel); B=Ln(expm-1) (fused -1 bias via a
    [128,1] const column, no DVE subtract); lnR=B+A via STT with accum_out;
    pos-masked sum via a second STT accum.
  - the unpaired class 18 runs per-quarter; in the last quarter it and the
    tail run in 512-wide chunks so the post-last-DMA drain is short.
Target is converted to bf16 on HOST (1MB instead of 2MB int32 DMA, no
on-chip CAST, and the first predict tile lands sooner).
Activation tables are pinned to natural_log_exp_and_others (holds both
ln and exp) -- otherwise bacc's table-load pass alternates between the
ln-only and exp-only sets, paying ~1.3us per reload.
Counts (pos/neg) are computed on host from the int target directly.
Per-core output is the raw [128, 16] per-partition stats; the final
partition reduce + 8-way combine happens on host in float64.
"""

from contextlib import ExitStack

import numpy as np

import concourse.bass as bass
import concourse.mybir as mybir
import concourse.tile as tile
from concourse import bacc
from concourse.bass_utils import run_bass_kernel_spmd

# problem shape (hardcoded per harness contract)
N, C, H, W = 8, 19, 512, 1024
PIX = H * W          # 524288 pixels per core
P = 128              # partitions
FCOLS = PIX // P     # 4096 free columns when pixels laid out [128, 4096]
QW = FCOLS // 4      # 1024: quarter width
HQW = QW // 2        # 512: final-chunk / matmul width
NPAIR = C // 2       # 9 class pairs; class 18 is the unpaired tail class
N_CORES = 8
NSTAT = 16           # stats columns in the [128, 16] output

DT = mybir.dt

# stats column layout ([128, 16] f32; host folds):
#   0-2 : sum lnR      for quarters 0-2
#   3-4 : sum lnR      for quarter 3 chunks 0,1
#   8-10: sum pos*lnR  for quarters 0-2
#   11-12: sum pos*lnR for quarter 3 chunks 0,1
COL_LNR = 0
COL_POSLNR = 8

_ACT_TABLES_PATCHED = False


def _pin_act_table_set():
    """Restrict Ln/Exp to the natural_log_exp_and_others set so bacc's
    table-load pass emits a single ACT_TABLE_LOAD instead of thrashing
    between the ln-only and exp-only sets (~1.3us per reload).  Set
    indices must stay aligned with act_info.json, so every set entry is
    kept -- only the Ln/Exp membership of the other sets is dropped."""
    global _ACT_TABLES_PATCHED
    if _ACT_TABLES_PATCHED:
        return
    import concourse.bacc as bacc_mod

    orig = bacc_mod.get_activation_tables
    ln_exp = {mybir.ActivationFunctionType.Ln, mybir.ActivationFunctionType.Exp}

    def patched(arch):
        tables = orig(arch)
        return {
            name: (fns if name == "natural_log_exp_and_others" else fns - ln_exp)
            for name, fns in tables.items()
        }

    bacc_mod.get_activation_tables = patched
    _ACT_TABLES_PATCHED = True


def build_kernel() -> bass.Bass:
    _pin_act_table_set()

    # Bacc (not raw Bass): its compile() pipeline runs
    # generate_event_semaphores, which splits multi-sem waits to satisfy the
    # 1-wait-per-instruction TRN2 sync structs -- raw Bass modules with
    # Tile-emitted multi-waits fail walrus codegen.
    nc = bacc.Bacc("TRN2")

    predict = nc.declare_dram_parameter("predict", [C, PIX], DT.float32, isOutput=False)
    target = nc.declare_dram_parameter("target", [P, FCOLS], DT.bfloat16, isOutput=False)
    idn = nc.declare_dram_parameter("idn", [P, P], DT.bfloat16, isOutput=False)
    out = nc.declare_dram_parameter("out", [P, NSTAT], DT.float32, isOutput=True)

    pred_r = predict.rearrange("c (p f) -> c p f", p=P)  # [19, 128, 4096]
    pred_t = predict.rearrange("c (p f) -> p c f", p=P)  # [128, 19, 4096]

    with tile.TileContext(nc) as tc, ExitStack() as ctx:
        const = ctx.enter_context(tc.tile_pool(name="const", bufs=1))
        # p bufs=8 aligns slot reuse with the global DMA->DMAHW-proc
        # round-robin (8 procs), so the WAW on the old writer is same-proc
        # FIFO order and Tile emits no cross-queue wait
        p_pool = ctx.enter_context(tc.tile_pool(name="p", bufs=8))
        lm_pool = ctx.enter_context(tc.tile_pool(name="lm", bufs=4))
        eq_pool = ctx.enter_context(tc.tile_pool(name="eq", bufs=4))
        # class-18 (unpaired) tiles are smaller and only 1 per quarter
        p18_pool = ctx.enter_context(tc.tile_pool(name="p18", bufs=4))
        lm18_pool = ctx.enter_context(tc.tile_pool(name="lm18", bufs=2))
        eq18_pool = ctx.enter_context(tc.tile_pool(name="eq18", bufs=2))
        tail_pool = ctx.enter_context(tc.tile_pool(name="tail", bufs=2))
        psA_pool = ctx.enter_context(tc.tile_pool(name="psA", bufs=2, space="PSUM"))
        psL_pool = ctx.enter_context(tc.tile_pool(name="psL", bufs=2, space="PSUM"))

        t_bf = const.tile([P, FCOLS], DT.bfloat16, tag="tb")
        # quarter 0 of target first so the q0 eq chain is ready before p0
        nc.sync.dma_start(out=t_bf[:, 0:QW], in_=target[:, 0:QW])

        idn_sb = const.tile([P, P], DT.bfloat16, tag="idn")
        stats = const.tile([P, NSTAT], DT.float32, tag="stats")
        # per-partition -1.0 bias column for the fused Ln(expm - 1) tail
        negone = const.tile([P, 1], DT.float32, tag="negone")

        state = {"n_dma": 0}

        def post_pair_dma():
            # constants queue behind pair0's DMAs; the bulk of target
            # queues behind pair1 -- the q0 pipeline primes first
            state["n_dma"] += 1
            if state["n_dma"] == 1:
                nc.sync.dma_start(out=idn_sb[:], in_=idn[:])
                nc.vector.memset(stats[:], 0.0)
                nc.vector.memset(negone[:], -1.0)
            elif state["n_dma"] == 2:
                nc.sync.dma_start(out=t_bf[:, QW:], in_=target[:, QW:])

        def tail_ops(q, a_ps, l_ps, toff, twidth, col):
            # tail: B = Ln(e^{-lsel} - 1) (bias fuses the -1); lnR = B + A.
            # Returned as 4 thunks so the caller can spread them across pair
            # slots of the NEXT quarter: ACT and DVE are in-order engines,
            # so an op must only be emitted once its producer ran >=1 pair
            # earlier, else the whole engine stream stalls behind it.
            qsl = slice(toff, toff + twidth)
            expm = tail_pool.tile([P, QW], DT.float32, tag="expm")
            bb = tail_pool.tile([P, QW], DT.float32, tag="bb")
            lnr = tail_pool.tile([P, QW], DT.float32, tag="lnr")
            scr = tail_pool.tile([P, QW], DT.float32, tag="scr")

            def op_exp():
                nc.scalar.activation(
                    out=expm[:, :twidth],
                    in_=l_ps[:, qsl],
                    func=mybir.ActivationFunctionType.Exp,
                    scale=-1.0,
                )

            def op_lnb():
                nc.scalar.activation(
                    out=bb[:, :twidth],
                    in_=expm[:, :twidth],
                    func=mybir.ActivationFunctionType.Ln,
                    bias=negone[:],
                )

            def op_lnr():
                nc.vector.scalar_tensor_tensor(
                    out=lnr[:, :twidth],
                    in0=bb[:, :twidth],
                    scalar=0.0,
                    in1=a_ps[:, qsl],
                    op0=mybir.AluOpType.add,
                    op1=mybir.AluOpType.add,
                    accum_out=stats[:, COL_LNR + col : COL_LNR + col + 1],
                )

            def op_scr():
                nc.vector.scalar_tensor_tensor(
                    out=scr[:, :twidth],
                    in0=t_bf[:, q * QW + toff : q * QW + toff + twidth],
                    scalar=0.5,
                    in1=lnr[:, :twidth],
                    op0=mybir.AluOpType.is_gt,
                    op1=mybir.AluOpType.mult,
                    accum_out=stats[:, COL_POSLNR + col : COL_POSLNR + col + 1],
                )

            return [op_exp, op_lnb, op_lnr, op_scr]

        # tail(q) thunks are spread across pairs 1-4 of quarter q+1
        pending_ops = []

        for q in range(4):
            qbase = q * QW
            qsl_full = slice(qbase, qbase + QW)
            # PSUM accumulators for this quarter (ping-pong, 2+2 banks each)
            a_ps = psA_pool.tile([P, QW], DT.float32, tag="aps")
            l_ps = psL_pool.tile([P, QW], DT.float32, tag="lps")

            for pair in range(NPAIR):
                c = 2 * pair
                # two plain 2D DMAs (128 descriptors each) into one tile:
                # a single 3D [p, c, f] DMA costs ~2.2us of serial
                # descriptor-generation on the sync sequencer vs ~0.6us x2
                p_t = p_pool.tile([P, 2, QW], DT.float32, tag="p")
                nc.sync.dma_start(out=p_t[:, 0, :], in_=pred_r[c, :, qsl_full])
                post_pair_dma()
                nc.sync.dma_start(out=p_t[:, 1, :], in_=pred_r[c + 1, :, qsl_full])
                post_pair_dma()

                # lm[:, 0:2, :] = L = Ln(1-p) bf16 ; lm[:, 2:4, :] = (T==c)*L
                lm = lm_pool.tile([P, 4, QW], DT.bfloat16, tag="lm")
                nc.scalar.activation(
                    out=lm[:, 0:2, :],
                    in_=p_t[:, :, :],
                    func=mybir.ActivationFunctionType.Ln,
                    bias=1.0,
                    scale=-1.0,
                )
                # eq at DVE 4x (16-bit tensor_scalar) + mult at 2x beats
                # the fused scalar_tensor_tensor, which only has a 1x uop
                eq = eq_pool.tile([P, 2, QW], DT.bfloat16, tag="eq")
                for j in range(2):
                    nc.vector.tensor_scalar(
                        out=eq[:, j, :],
                        in0=t_bf[:, qsl_full],
                        scalar1=float(c + j),
                        scalar2=None,
                        op0=mybir.AluOpType.is_equal,
                    )
                nc.vector.tensor_mul(
                    out=lm[:, 2:4, :],
                    in0=eq[:, 0:2, :],
                    in1=lm[:, 0:2, :],
                )

                # lsel matmuls first: l_ps frees early in the tail (Exp
                # is its only reader), so the next quarter's PE work
                # restarts sooner
                for mrow, which in ((2, "l"), (3, "l"), (0, "a"), (1, "a")):
                    dst_ps = l_ps if which == "l" else a_ps
                    cc = c + (mrow % 2)
                    for s in range(2):
                        nc.tensor.matmul(
                            dst_ps[:, s * HQW : (s + 1) * HQW],
                            lhsT=idn_sb[:],
                            rhs=lm[:, mrow, s * HQW : (s + 1) * HQW],
                            start=(cc == 0),
                            stop=False,
                        )

                if pair >= 1 and pending_ops:
                    pending_ops.pop(0)()

            # unpaired class 18; split into two 512 chunks in the last
            # quarter so the tail can start on chunk 0 while chunk 1
            # still computes
            chunks = [(s * HQW, HQW) for s in range(2)] if q == 3 else [(0, QW)]
            for off, width in chunks:
                csl = slice(qbase + off, qbase + off + width)
                p_s = p18_pool.tile([P, QW], DT.float32, tag="p18")
                nc.sync.dma_start(out=p_s[:, :width], in_=pred_r[C - 1, :, csl])
                lm = lm18_pool.tile([P, 2 * QW], DT.bfloat16, tag="lm18")
                nc.scalar.activation(
                    out=lm[:, :width],
                    in_=p_s[:, :width],
                    func=mybir.ActivationFunctionType.Ln,
                    bias=1.0,
                    scale=-1.0,
                )
                eq = eq18_pool.tile([P, QW], DT.bfloat16, tag="eq18")
                nc.vector.tensor_scalar(
                    out=eq[:, :width],
                    in0=t_bf[:, csl],
                    scalar1=float(C - 1),
                    scalar2=None,
                    op0=mybir.AluOpType.is_equal,
                )
                nc.vector.tensor_mul(
                    out=lm[:, QW : QW + width],
                    in0=eq[:, :width],
                    in1=lm[:, :width],
                )
                for s in range(width // HQW):
                    nc.tensor.matmul(
                        l_ps[:, off + s * HQW : off + (s + 1) * HQW],
                        lhsT=idn_sb[:],
                        rhs=lm[:, QW + s * HQW : QW + (s + 1) * HQW],
                        start=False,
                        stop=True,
                    )
                for s in range(width // HQW):
                    nc.tensor.matmul(
                        a_ps[:, off + s * HQW : off + (s + 1) * HQW],
                        lhsT=idn_sb[:],
                        rhs=lm[:, s * HQW : (s + 1) * HQW],
                        start=False,
                        stop=True,
                    )

            if q < 3:
                pending_ops = tail_ops(q, a_ps, l_ps, 0, QW, q)

        # final quarter's tail runs at the very end, in two 512-wide chunks
        # so the post-last-DMA latency chain is short
        for ci in range(2):
            for op in tail_ops(3, a_ps, l_ps, ci * HQW, HQW, 3 + ci):
                op()

        nc.sync.dma_start(out=out[:], in_=stats[:])

    if not nc.is_finalized():
        nc.finalize()

    return nc


_NC_CACHE = None


def make_in_maps(predict: np.ndarray, target: np.ndarray):
    import ml_dtypes

    predict = np.ascontiguousarray(predict, dtype=np.float32)
    target_bf = np.ascontiguousarray(target, dtype=np.int32).astype(ml_dtypes.bfloat16)
    idn = np.eye(P, dtype=np.float32).astype(ml_dtypes.bfloat16)

    in_maps = []
    for k in range(N_CORES):
        in_maps.append(
            {
                "predict": predict[k].reshape(C, PIX),
                "target": target_bf[k].reshape(P, FCOLS),
                "idn": idn,
            }
        )
    return in_maps


def combine_host(results, target: np.ndarray) -> np.float32:
    tot = np.float64(0.0)
    s_all = np.float64(0.0)
    s_pos = np.float64(0.0)
    for k in range(N_CORES):
        st = results[k]["out"].reshape(P, NSTAT).astype(np.float64)
        s_all += -np.sum(st[:, COL_LNR : COL_LNR + 5])
        s_pos += -np.sum(st[:, COL_POSLNR : COL_POSLNR + 5])
        tot += PIX
    pos = np.float64(np.count_nonzero(target))
    neg = tot - pos
    s_neg = s_all - s_pos
    loss = ((neg / tot) * s_pos + (pos / tot) * s_neg) / (tot * C)
    return np.float32(loss)


def kernel(predict: np.ndarray, target: np.ndarray) -> np.ndarray:
    global _NC_CACHE
    if _NC_CACHE is None:
        _NC_CACHE = build_kernel()
    nc = _NC_CACHE

    in_maps = make_in_maps(predict, target)
    res = run_bass_kernel_spmd(nc, in_maps, list(range(N_CORES)))
    return combine_host(res.results, target)


# revision 21
# speedup vs baseline: 1.1258x; 1.0004x over previous
"""Weighted 2D cross-entropy (BCE-over-classes) loss on 8 Trainium2 cores.

Math (matches the reference):
  t in [0,19); pos = t>0, neg = t==0 (all pixels are pos or neg; mask == 1)
  S(i) = sum_c bce(i,c) = -lnR(i)
     lnR(i) = A(i) + B(i)
     A(i)   = sum_c ln(1-p_c(i))
     B(i)   = ln(p_t(i)) - ln(1-p_t(i)) = ln(e^{-lsel(i)} - 1),  lsel = ln(1-p_t)
  loss = ( (NEG/TOT)*S_pos_sum + (POS/TOT)*S_neg_sum ) / (TOT*C)

Per-core (core k <- batch element k, pure data parallel), FOUR quarter-passes
over pixel quarters [128, 1024].  A quarter's PSUM accumulators are four
single-bank [128, 512] half-tiles (A and lsel, each split in half), so two
quarters ping-pong in PSUM (8 banks total): while quarter q's tail
(Exp/Ln/STT chain on ACT+DVE) drains its banks, the PE already streams
quarter q+1's matmuls, and each 512-wide tail chunk depends only on its own
half's matmuls (whole-tile deps would chain it to the last matmul).

Schedule, tuned against the 1-wait-per-instruction TRN2 sync structs (a
multi-wait op gets its extra waits split onto PRECEDING ops of the same
in-order engine, stalling them):
  - quarter 0 opens with two single-DMA 512-wide class-18 chunks: their Ln
    needs one wait, so the ACT_TABLE_LOAD (first ACT-queue entry) stays
    wait-free and runs during the DMA ramp, and the first Ln starts on
    0.25MB instead of 1MB of data.
  - quarters 0-2 then process classes in PAIRS: one Ln / one mult covers
    [128, 2048], halving the ~294ns fixed per-op ACT/DVE overhead (ACT
    otherwise paces the DMA-bound stream).  Each pair is TWO plain 2D DMAs
    into one tile (a single 3D [p, c, f] DMA costs ~2.2us of serial
    descriptor-generation on the sync sequencer vs ~0.6us x2).
  - quarter 3 processes classes SINGLY (single-DMA 0.5MB units, class 18 in
    two 512 chunks): after the last DMA byte only a small unit remains in
    flight, so the end-drain is short.
  - tail per half: expm=Exp(-lsel); B=Ln(expm-1) (fused -1 bias via a
    [128,1] const column); lnR=B+A via STT with accum_out; pos-masked sum
    via a second STT accum.  tail(q) is deferred and its 8 ops spread one
    per unit across quarter q+1, each emitted >=1 unit after its producer
    so no in-order engine stream ever stalls behind it.
Target is converted to bf16 on HOST (1MB instead of 2MB int32 DMA, no
on-chip CAST, and the first predict tile lands sooner).
Activation tables are pinned to natural_log_exp_and_others (holds both
ln and exp) -- otherwise bacc's table-load pass alternates between the
ln-only and exp-only sets, paying ~1.3us per reload.
Counts (pos/neg) are computed on host from the int target directly.
Per-core output is the raw [128, 16] per-partition stats; the final
partition reduce + 8-way combine happens on host in float64.
"""

from contextlib import ExitStack

import numpy as np

import concourse.bass as bass
import concourse.mybir as mybir
import concourse.tile as tile
from concourse import bacc
from concourse.bass_utils import run_bass_kernel_spmd

# problem shape (hardcoded per harness contract)
N, C, H, W = 8, 19, 512, 1024
PIX = H * W          # 524288 pixels per core
P = 128              # partitions
FCOLS = PIX // P     # 4096 free columns when pixels laid out [128, 4096]
QW = FCOLS // 4      # 1024: quarter width
HQW = QW // 2        # 512: half-quarter (PSUM bank / matmul / tail width)
NPAIR = C // 2       # 9 class pairs; class 18 is the unpaired class
N_CORES = 8
NSTAT = 16           # stats columns in the [128, 16] output

DT = mybir.dt

# stats column layout ([128, 16] f32; host folds):
#   2q+h     : sum lnR      for quarter q, half h
#   8+2q+h   : sum pos*lnR  for quarter q, half h
COL_LNR = 0
COL_POSLNR = 8

_ACT_TABLES_PATCHED = False


def _pin_act_table_set():
    """Restrict Ln/Exp to the natural_log_exp_and_others set so bacc's
    table-load pass emits a single ACT_TABLE_LOAD instead of thrashing
    between the ln-only and exp-only sets (~1.3us per reload).  Set
    indices must stay aligned with act_info.json, so every set entry is
    kept -- only the Ln/Exp membership of the other sets is dropped."""
    global _ACT_TABLES_PATCHED
    if _ACT_TABLES_PATCHED:
        return
    import concourse.bacc as bacc_mod

    orig = bacc_mod.get_activation_tables
    ln_exp = {mybir.ActivationFunctionType.Ln, mybir.ActivationFunctionType.Exp}

    def patched(arch):
        tables = orig(arch)
        return {
            name: (fns if name == "natural_log_exp_and_others" else fns - ln_exp)
            for name, fns in tables.items()
        }

    bacc_mod.get_activation_tables = patched
    _ACT_TABLES_PATCHED = True


def build_kernel() -> bass.Bass:
    _pin_act_table_set()

    # Bacc (not raw Bass): its compile() pipeline runs
    # generate_event_semaphores, which splits multi-sem waits to satisfy the
    # 1-wait-per-instruction TRN2 sync structs -- raw Bass modules with
    # Tile-emitted multi-waits fail walrus codegen.
    nc = bacc.Bacc("TRN2")

    predict = nc.declare_dram_parameter("predict", [C, PIX], DT.float32, isOutput=False)
    target = nc.declare_dram_parameter("target", [P, FCOLS], DT.bfloat16, isOutput=False)
    idn = nc.declare_dram_parameter("idn", [P, P], DT.bfloat16, isOutput=False)
    out = nc.declare_dram_parameter("out", [P, NSTAT], DT.float32, isOutput=True)

    pred_r = predict.rearrange("c (p f) -> c p f", p=P)  # [19, 128, 4096]

    with tile.TileContext(nc) as tc, ExitStack() as ctx:
        const = ctx.enter_context(tc.tile_pool(name="const", bufs=1))
        # p bufs=8 aligns slot reuse with the global DMA->DMAHW-proc
        # round-robin (8 procs), so the WAW on the old writer is same-proc
        # FIFO order and Tile emits no cross-queue wait
        p_pool = ctx.enter_context(tc.tile_pool(name="p", bufs=8))
        lm_pool = ctx.enter_context(tc.tile_pool(name="lm", bufs=4))
        eq_pool = ctx.enter_context(tc.tile_pool(name="eq", bufs=4))
        # single-class (0.5MB) units: quarter-3 stream + the quarter-0 prime
        ps_pool = ctx.enter_context(tc.tile_pool(name="ps", bufs=6))
        lms_pool = ctx.enter_context(tc.tile_pool(name="lms", bufs=3))
        eqs_pool = ctx.enter_context(tc.tile_pool(name="eqs", bufs=3))
        tail_pool = ctx.enter_context(tc.tile_pool(name="tail", bufs=2))
        psAa_pool = ctx.enter_context(tc.tile_pool(name="psAa", bufs=2, space="PSUM"))
        psAb_pool = ctx.enter_context(tc.tile_pool(name="psAb", bufs=2, space="PSUM"))
        psLa_pool = ctx.enter_context(tc.tile_pool(name="psLa", bufs=2, space="PSUM"))
        psLb_pool = ctx.enter_context(tc.tile_pool(name="psLb", bufs=2, space="PSUM"))

        t_bf = const.tile([P, FCOLS], DT.bfloat16, tag="tb")
        # quarter 0 of target first so the q0 eq chain is ready before p0
        nc.sync.dma_start(out=t_bf[:, 0:QW], in_=target[:, 0:QW])

        idn_sb = const.tile([P, P], DT.bfloat16, tag="idn")
        stats = const.tile([P, NSTAT], DT.float32, tag="stats")
        # per-partition -1.0 bias column for the fused Ln(expm - 1) tail
        negone = const.tile([P, 1], DT.float32, tag="negone")

        state = {"n_dma": 0}

        def count_dma():
            # constants queue behind the first data DMA; the bulk of target
            # queues behind the second -- the q0 pipeline primes first
            state["n_dma"] += 1
            if state["n_dma"] == 1:
                nc.sync.dma_start(out=idn_sb[:], in_=idn[:])
                nc.vector.memset(stats[:], 0.0)
                nc.vector.memset(negone[:], -1.0)
            elif state["n_dma"] == 2:
                nc.sync.dma_start(out=t_bf[:, QW:], in_=target[:, QW:])

        def emit_single(q, c, off, width, halves, start, stop):
            # one class's [off, off+width) slice of quarter q: single DMA,
            # Ln, eq, mask-mult, then per-512 matmuls into the half tiles
            qbase = q * QW
            csl = slice(qbase + off, qbase + off + width)
            p_s = ps_pool.tile([P, QW], DT.float32, tag="ps")
            nc.sync.dma_start(out=p_s[:, :width], in_=pred_r[c, :, csl])
            count_dma()
            lm = lms_pool.tile([P, 2 * QW], DT.bfloat16, tag="lms")
            nc.scalar.activation(
                out=lm[:, :width],
                in_=p_s[:, :width],
                func=mybir.ActivationFunctionType.Ln,
                bias=1.0,
                scale=-1.0,
            )
            eq = eqs_pool.tile([P, QW], DT.bfloat16, tag="eqs")
            nc.vector.tensor_scalar(
                out=eq[:, :width],
                in0=t_bf[:, csl],
                scalar1=float(c),
                scalar2=None,
                op0=mybir.AluOpType.is_equal,
            )
            nc.vector.tensor_mul(
                out=lm[:, QW : QW + width],
                in0=eq[:, :width],
                in1=lm[:, :width],
            )
            for s in range(width // HQW):
                h = (off + s * HQW) // HQW
                l_h, a_h = halves[h]
                nc.tensor.matmul(
                    l_h[:, :],
                    lhsT=idn_sb[:],
                    rhs=lm[:, QW + s * HQW : QW + (s + 1) * HQW],
                    start=start,
                    stop=stop,
                )
            for s in range(width // HQW):
                h = (off + s * HQW) // HQW
                l_h, a_h = halves[h]
                nc.tensor.matmul(
                    a_h[:, :],
                    lhsT=idn_sb[:],
                    rhs=lm[:, s * HQW : (s + 1) * HQW],
                    start=start,
                    stop=stop,
                )

        def emit_pair(q, c, halves, start, stop):
            qbase = q * QW
            qsl_full = slice(qbase, qbase + QW)
            # two plain 2D DMAs (128 descriptors each) into one tile
            p_t = p_pool.tile([P, 2, QW], DT.float32, tag="p")
            nc.sync.dma_start(out=p_t[:, 0, :], in_=pred_r[c, :, qsl_full])
            count_dma()
            nc.sync.dma_start(out=p_t[:, 1, :], in_=pred_r[c + 1, :, qsl_full])
            count_dma()

            # lm[:, 0:2, :] = L = Ln(1-p) bf16 ; lm[:, 2:4, :] = (T==c)*L
            lm = lm_pool.tile([P, 4, QW], DT.bfloat16, tag="lm")
            nc.scalar.activation(
                out=lm[:, 0:2, :],
                in_=p_t[:, :, :],
                func=mybir.ActivationFunctionType.Ln,
                bias=1.0,
                scale=-1.0,
            )
            # eq at DVE 4x (16-bit tensor_scalar) + mult at 2x beats
            # the fused scalar_tensor_tensor, which only has a 1x uop
            eq = eq_pool.tile([P, 2, QW], DT.bfloat16, tag="eq")
            for j in range(2):
                nc.vector.tensor_scalar(
                    out=eq[:, j, :],
                    in0=t_bf[:, qsl_full],
                    scalar1=float(c + j),
                    scalar2=None,
                    op0=mybir.AluOpType.is_equal,
                )
            nc.vector.tensor_mul(
                out=lm[:, 2:4, :],
                in0=eq[:, 0:2, :],
                in1=lm[:, 0:2, :],
            )

            # lsel matmuls first: the lsel halves free first in the tail
            # (Exp is their only reader), so the next quarter's PE work
            # restarts sooner
            # start belongs to the group-opening class (c==0) only; stop to
            # the group-closing one (the pair's second class) only
            for mrow, which in ((2, "l"), (3, "l"), (0, "a"), (1, "a")):
                second = mrow % 2 == 1
                for s in range(2):
                    l_h, a_h = halves[s]
                    dst = l_h if which == "l" else a_h
                    nc.tensor.matmul(
                        dst[:, :],
                        lhsT=idn_sb[:],
                        rhs=lm[:, mrow, s * HQW : (s + 1) * HQW],
                        start=start and not second,
                        stop=stop and second,
                    )

        def tail_ops(q, h, l_h, a_h):
            # tail for half h of quarter q: B = Ln(e^{-lsel} - 1) (bias
            # fuses the -1); lnR = B + A; two accumulating STTs.
            # Returned as 4 thunks so the caller can spread them across the
            # NEXT quarter's units: ACT and DVE are in-order engines, so an
            # op must only be emitted once its producer ran >=1 unit
            # earlier, else the whole engine stream stalls behind it.
            col = 2 * q + h
            toff = q * QW + h * HQW
            expm = tail_pool.tile([P, HQW], DT.float32, tag=f"expm{h}")
            bb = tail_pool.tile([P, HQW], DT.float32, tag=f"bb{h}")
            lnr = tail_pool.tile([P, HQW], DT.float32, tag=f"lnr{h}")
            scr = tail_pool.tile([P, HQW], DT.float32, tag=f"scr{h}")

            def op_exp():
                nc.scalar.activation(
                    out=expm[:, :],
                    in_=l_h[:, :],
                    func=mybir.ActivationFunctionType.Exp,
                    scale=-1.0,
                )

            def op_lnb():
                nc.scalar.activation(
                    out=bb[:, :],
                    in_=expm[:, :],
                    func=mybir.ActivationFunctionType.Ln,
                    bias=negone[:],
                )

            def op_lnr():
                nc.vector.scalar_tensor_tensor(
                    out=lnr[:, :],
                    in0=bb[:, :],
                    scalar=0.0,
                    in1=a_h[:, :],
                    op0=mybir.AluOpType.add,
                    op1=mybir.AluOpType.add,
                    accum_out=stats[:, COL_LNR + col : COL_LNR + col + 1],
                )

            def op_scr():
                nc.vector.scalar_tensor_tensor(
                    out=scr[:, :],
                    in0=t_bf[:, toff : toff + HQW],
                    scalar=0.5,
                    in1=lnr[:, :],
                    op0=mybir.AluOpType.is_gt,
                    op1=mybir.AluOpType.mult,
                    accum_out=stats[:, COL_POSLNR + col : COL_POSLNR + col + 1],
                )

            return [op_exp, op_lnb, op_lnr, op_scr]

        # tail(q)'s 8 thunks are spread one per unit across quarter q+1,
        # starting at its second unit
        pending_ops = []

        for q in range(4):
            # PSUM half accumulators for this quarter (ping-pong, 1 bank each)
            halves = []
            for h, (lp, ap) in enumerate(
                ((psLa_pool, psAa_pool), (psLb_pool, psAb_pool))
            ):
                l_h = lp.tile([P, HQW], DT.float32, tag=f"l{h}")
                a_h = ap.tile([P, HQW], DT.float32, tag=f"a{h}")
                halves.append((l_h, a_h))

            def unit_boundary():
                if pending_ops:
                    pending_ops.pop(0)()

            if q == 0:
                # prime: two single-DMA 512 chunks of class 18 (single-wait
                # Lns keep the ACT table load free to run during the ramp)
                emit_single(q, C - 1, 0, HQW, halves, start=True, stop=False)
                emit_single(q, C - 1, HQW, HQW, halves, start=True, stop=False)
                for pair in range(NPAIR):
                    if pair >= 1:
                        unit_boundary()
                    emit_pair(q, 2 * pair, halves, start=False,
                              stop=(pair == NPAIR - 1))
            elif q < 3:
                for pair in range(NPAIR):
                    if pair >= 1:
                        unit_boundary()
                    emit_pair(q, 2 * pair, halves, start=(pair == 0), stop=False)
                unit_boundary()
                emit_single(q, C - 1, 0, QW, halves, start=False, stop=True)
            else:
                # last quarter: single-class units so the end-drain after the
                # final (small) DMA is short
                for c in range(C - 1):
                    if c >= 1:
                        unit_boundary()
                    emit_single(q, c, 0, QW, halves, start=(c == 0), stop=False)
                emit_single(q, C - 1, 0, HQW, halves, start=False, stop=True)
                emit_single(q, C - 1, HQW, HQW, halves, start=False, stop=True)

            if q < 3:
                pending_ops = tail_ops(q, 0, *halves[0]) + tail_ops(q, 1, *halves[1])
            else:
                # final tails inline, interleaved a/b for minimal latency
                ta = tail_ops(q, 0, halves[0][0], halves[0][1])
                tb = tail_ops(q, 1, halves[1][0], halves[1][1])
                for op in (ta[0], ta[1], tb[0], tb[1], ta[2], ta[3], tb[2], tb[3]):
                    op()

        nc.sync.dma_start(out=out[:], in_=stats[:])

    if not nc.is_finalized():
        nc.finalize()

    return nc


_NC_CACHE = None


def make_in_maps(predict: np.ndarray, target: np.ndarray):
    import ml_dtypes

    predict = np.ascontiguousarray(predict, dtype=np.float32)
    target_bf = np.ascontiguousarray(target, dtype=np.int32).astype(ml_dtypes.bfloat16)
    idn = np.eye(P, dtype=np.float32).astype(ml_dtypes.bfloat16)

    in_maps = []
    for k in range(N_CORES):
        in_maps.append(
            {
                "predict": predict[k].reshape(C, PIX),
                "target": target_bf[k].reshape(P, FCOLS),
                "idn": idn,
            }
        )
    return in_maps


def combine_host(results, target: np.ndarray) -> np.float32:
    tot = np.float64(0.0)
    s_all = np.float64(0.0)
    s_pos = np.float64(0.0)
    for k in range(N_CORES):
        st = results[k]["out"].reshape(P, NSTAT).astype(np.float64)
        s_all += -np.sum(st[:, COL_LNR : COL_LNR + 8])
        s_pos += -np.sum(st[:, COL_POSLNR : COL_POSLNR + 8])
        tot += PIX
    pos = np.float64(np.count_nonzero(target))
    neg = tot - pos
    s_neg = s_all - s_pos
    loss = ((neg / tot) * s_pos + (pos / tot) * s_neg) / (tot * C)
    return np.float32(loss)


def kernel(predict: np.ndarray, target: np.ndarray) -> np.ndarray:
    global _NC_CACHE
    if _NC_CACHE is None:
        _NC_CACHE = build_kernel()
    nc = _NC_CACHE

    in_maps = make_in_maps(predict, target)
    res = run_bass_kernel_spmd(nc, in_maps, list(range(N_CORES)))
    return combine_host(res.results, target)
